# revision 37
# baseline (speedup 1.0000x reference)
"""Trainium2 Bass kernel for EnhanceLayerLinear.

Computes out = GroupedLinear(Linear(x)):
    y = x @ W.T + b                      [B,S,D]
    out[..., g, :] = y[..., g, :] @ Wg[g].T + bg[g]   (block-diagonal, G groups)

Sharding: data-parallel over tokens (B*S = 8192 -> 1024 per core). Each core
runs both GEMM stages locally; the grouped stage shards trivially since it is
applied per token.

Mixed-precision stage 1: the 4096-wide contraction is split into KB=22 bf16
k-tiles plus UF8=5 fp8e4 DoubleRow units (2 k-tiles each). A DoubleRow matmul
contracts K=256 (two k-planes packed per partition: lhsT [128,2,128], rhs
[128,2,512] -> out [128,512]) in the same ~214 ns slot as one bf16 K=128 MM
-- 2x the MAC rate, measured in-kernel -- halving stage-1 PE time on the
covered fraction. fp8 coverage is capped by the 2e-2 error gate: 10/32 of
the contraction in e4m3 (both operands) measures 1.53e-2 absmax-rel /
1.78e-2 L2 vs the fp32 reference on the full output.

fp8 scaling: W is pre-scaled by 64 before quantization (sigma(64W)=1.28 sits
mid-range in e4m3; unscaled W would land 56% of weights in subnormals). The
bf16 W tiles carry the same 2^6 factor (exact in bf16) so both dtypes
accumulate into ONE psum tile per og-pass (start on the pass's first MM,
stop on its last, skip_group_check elsewhere), and the psum evacuation
applies scale=2^-6 for free via the activation's scale operand. Pass block
order alternates by parity ([bf16,fp8] / [fp8,bf16]) so adjacent passes'
DoubleRow blocks run back-to-back, halving bf16<->DoubleRow mode switches.

Stage 2 runs in bf16 (y evacuated as bf16, Wg bf16): 213 ns/slot vs f32r's
422 ns (2-pass fp32 LDWEIGHTS cannot hide); the added y-rounding error is
~2.6e-3, inside the budget.

Layout trick (from the bf16 baseline): stage 1 computes y TRANSPOSED
(features on partitions, tokens on the free axis) so each 128-row psum tile
is exactly one group's slice with stage 2's contraction already on
partitions; the grouped matmul chains with zero on-chip transposes. The host
hands the kernel pre-transposed views of x / W / Wg and re-transposes the
output.
"""

from collections import deque

import ml_dtypes
import numpy as np

import concourse.bacc as bacc
import concourse.bass as bass
import concourse.tile as tile
from concourse import mybir
from concourse import bass_utils

f32 = mybir.dt.float32
bf16 = mybir.dt.bfloat16
fp8 = mybir.dt.float8e4
DR = mybir.MatmulPerfMode.DoubleRow
ACT_ID = mybir.ActivationFunctionType.Identity

B, S, D = 4, 2048, 4096
T = B * S                 # 8192 tokens
G, IG = 32, 128           # groups x group size (4096 = 32*128)
NCORES = 8
TPC = T // NCORES         # 1024 tokens per core
KT = D // 128             # 32 contraction tiles
UF8 = 5                   # fp8 DoubleRow units (2 k-tiles each)
KB = KT - 2 * UF8         # bf16 k-tiles
NMOV = 512                # moving free dim per bf16 matmul (= one psum bank)
NCH = TPC // NMOV         # 2 token chunks per core
WSCALE = 64.0             # fp8 weight pre-scale (power of 2)

_CACHE = {}


def _build():
    nc = bacc.Bacc("TRN2", target_bir_lowering=False, debug=False)
    # x_d[kt, tch, p, t]    = x[core_t0 + tch*512 + t, kt*128 + p]    (bf16 kt)
    # x8_d[u, tch, p, j, t] = e4m3 of x[..., (KB+2u+j)*128 + p]
    # w_d[og, p, kt*128+o]  = bf16(64*W[og*128+o, kt*128+p])
    # w8_d[og, p, u, j, m]  = e4m3(64*W[og*128+m, (KB+2u+j)*128+p])
    # wg_d[i, g*128+o] = Wg[g, o, i];  b_d[i, g] = b[g*128+i];  bg_d[o, g]
    x_d = nc.dram_tensor("x", [KB, NCH, 128, NMOV], bf16, kind="ExternalInput")
    x8_d = nc.dram_tensor(
        "x8", [UF8, NCH, 128, 2, NMOV], fp8, kind="ExternalInput"
    )
    w_d = nc.dram_tensor("w", [G, 128, KB * 128], bf16, kind="ExternalInput")
    w8_d = nc.dram_tensor("w8", [G, 128, UF8, 2, 128], fp8, kind="ExternalInput")
    wg_d = nc.dram_tensor("wg", [128, G * IG], bf16, kind="ExternalInput")
    b_d = nc.dram_tensor("b", [128, G], f32, kind="ExternalInput")
    bg_d = nc.dram_tensor("bg", [128, G], f32, kind="ExternalInput")
    # o_d[og, o, t] = out[core_t0 + t, og*128 + o]                (outT)
    o_d = nc.dram_tensor("o", [G, 128, TPC], f32, kind="ExternalOutput")

    with tile.TileContext(nc) as tc:
        with (
            tc.tile_pool(name="xp", bufs=KB * NCH) as xp,
            tc.tile_pool(name="x8p", bufs=UF8 * NCH) as x8p,
            tc.tile_pool(name="wp", bufs=6) as wp,
            tc.tile_pool(name="w8p", bufs=6) as w8p,
            tc.tile_pool(name="cp", bufs=1) as cp,
            tc.tile_pool(name="yp", bufs=18) as yp,
            tc.tile_pool(name="op", bufs=6) as op,
            tc.tile_pool(name="ps1", bufs=4, space=bass.MemorySpace.PSUM) as ps1,
            tc.tile_pool(name="ps2", bufs=4, space=bass.MemorySpace.PSUM) as ps2,
        ):
            w_tiles = {}
            w8_tiles = {}

            # DMA queue split (queue = issuing engine): x wave rides the sync
            # HW-DGE queue, the W stream rides gpsimd's queue, outputs and
            # consts ride scalar's -- three streams in parallel instead of
            # one serialized sync queue.
            def load_w(key):
                t = wp.tile([128, KB * 128], bf16, tag="w")
                nc.gpsimd.dma_start(t[:], w_d[key[1]])
                w_tiles[key] = t
                t8 = w8p.tile([128, UF8, 2, 128], fp8, tag="w8")
                nc.gpsimd.dma_start(t8[:], w8_d[key[1]])
                w8_tiles[key] = t8

            # The first ~25us is DMA-bandwidth-bound, so queue order here IS
            # the schedule. The first RAMP groups run INTERLEAVED (kt-major
            # across RAMP psum banks) so each arriving x tile feeds RAMP
            # matmuls and the PE stays busy through the whole x wave; their W
            # tiles are delivered as just-in-time column chunks between the x
            # tiles they gate. The fp8 halves of the ramp groups run last,
            # fed by the small x8/w8 tiles queued behind the bf16 x wave.
            RAMP = 4
            b_sb = cp.tile([128, G], f32)
            nc.scalar.dma_start(b_sb[:], b_d[:])
            ramp_w = []
            ramp_w8 = []
            for og in range(RAMP):
                t = wp.tile([128, KB * 128], bf16, tag="w")
                ramp_w.append(t)
                w_tiles[(0, og)] = t
                t8 = w8p.tile([128, UF8, 2, 128], fp8, tag="w8")
                ramp_w8.append(t8)
                w8_tiles[(0, og)] = t8
            x_sb = [[None] * NCH for _ in range(KB)]
            x8_sb = [[None] * NCH for _ in range(UF8)]
            wg_sb = cp.tile([128, G * IG], bf16)
            bg_sb = cp.tile([128, G], f32)
            # The chunk-0 x wave is DMA-POST-rate limited (~0.73us per post,
            # serialized per engine) against the PE's 0.856us/tile ramp
            # consumption, so alternate the posts between the sync and scalar
            # HW-DGE queues -- scalar's queue is idle until the first output
            # DMA at ~48us.
            WCHUNKS = [1, 5, 5, 6, 5]
            kt0 = 0
            for wc in WCHUNKS:
                lo, hi = kt0 * 128, (kt0 + wc) * 128
                for og in range(RAMP):
                    nc.gpsimd.dma_start(ramp_w[og][:, lo:hi], w_d[og][:, lo:hi])
                for kt in range(kt0, kt0 + wc):
                    t = xp.tile([128, NMOV], bf16, tag="x")
                    eng = nc.sync if kt % 2 == 0 else nc.scalar
                    eng.dma_start(t[:], x_d[kt, 0])
                    x_sb[kt][0] = t
                kt0 += wc
            for og in range(RAMP):
                nc.gpsimd.dma_start(ramp_w8[og][:], w8_d[og])
            for u in range(UF8):
                t = x8p.tile([128, 2, NMOV], fp8, tag="x8")
                eng = nc.sync if u % 2 == 0 else nc.scalar
                eng.dma_start(t[:], x8_d[u, 0])
                x8_sb[u][0] = t
            load_w((0, RAMP))
            load_w((0, RAMP + 1))
            load_w((0, RAMP + 2))
            nc.scalar.dma_start(wg_sb[:], wg_d[:])
            nc.scalar.dma_start(bg_sb[:], bg_d[:])

            pending_q = deque()
            FLUSH_LAG = 6

            def flush_stage2(p):
                y_sb, og2, tch2 = p
                acc2 = ps2.tile([128, NMOV], f32, tag="acc2")
                nc.tensor.matmul(
                    acc2[:],
                    wg_sb[:, og2 * IG:(og2 + 1) * IG],
                    y_sb[:],
                    start=True,
                    stop=True,
                )
                o_sb = op.tile([128, NMOV], f32, tag="o")
                nc.scalar.activation(
                    o_sb[:], acc2[:], ACT_ID, bias=bg_sb[:, og2:og2 + 1]
                )
                nc.scalar.dma_start(
                    o_d[og2][:, tch2 * NMOV:(tch2 + 1) * NMOV], o_sb[:]
                )

            def fp8_mms(acc, w8_sb, tch):
                # One DoubleRow MM per unit: rhs free = 2x512 -> out free 512,
                # so the fp8 MMs write the same full psum width as the bf16
                # ones, at ~214 ns each vs ~428 ns for the bf16 k-tile PAIR
                # they replace (2x MAC rate, measured in-kernel). The block
                # sits strictly interior to the pass (start/stop both live on
                # bf16 k-tiles).
                for u in range(UF8):
                    nc.tensor.matmul(
                        acc[:],
                        w8_sb[:, u],
                        x8_sb[u][tch][:],
                        start=False,
                        stop=False,
                        perf_mode=DR,
                        skip_group_check=True,
                    )

            def bf16_mms(acc, w_sb, tch, lo, hi):
                for kt in range(lo, hi):
                    nc.tensor.matmul(
                        acc[:],
                        w_sb[:, kt * 128:(kt + 1) * 128],
                        x_sb[kt][tch][:],
                        start=(kt == 0),
                        stop=(kt == KB - 1),
                        skip_group_check=True,
                    )

            def evac_y(acc, og):
                y_sb = yp.tile([128, NMOV], bf16, tag="y")
                nc.scalar.activation(
                    y_sb[:], acc[:], ACT_ID,
                    bias=b_sb[:, og:og + 1], scale=1.0 / WSCALE,
                )
                return y_sb

            # Interleaved ramp: RAMP accumulation groups advance together,
            # kt-major, one psum bank each, paced by the x-tile arrivals;
            # every group's fp8 block runs at the end (one mode switch for
            # the whole ramp), fed by the small x8/w8 tiles queued behind
            # the bf16 x wave.
            accs = []
            for _r in range(RAMP):
                acc_r = ps1.tile([128, NMOV], f32, tag="acc")
                accs.append(acc_r)
            for kt in range(KB - 1):
                for og in range(RAMP):
                    nc.tensor.matmul(
                        accs[og][:],
                        ramp_w[og][:, kt * 128:(kt + 1) * 128],
                        x_sb[kt][0][:],
                        start=(kt == 0),
                        stop=False,
                        skip_group_check=True,
                    )
            for og in range(RAMP):
                fp8_mms(accs[og], ramp_w8[og], 0)
                nc.tensor.matmul(
                    accs[og][:],
                    ramp_w[og][:, (KB - 1) * 128:KB * 128],
                    x_sb[KB - 1][0][:],
                    start=False,
                    stop=True,
                    skip_group_check=True,
                )
                pending_q.append((evac_y(accs[og], og), og, 0))

            # tch outer: the whole first token-chunk pass (32 groups) runs
            # before any tch=1 tile is needed, so the second x wave has
            # enormous DMA slack. W streams twice; still far below the
            # per-core HBM budget.
            x2q = [("x", kt) for kt in range(KB)] + [("x8", u) for u in range(UF8)]
            passes = [(tch, og) for tch in range(NCH) for og in range(G)]
            for idx in range(RAMP, len(passes)):
                tch, og = passes[idx]
                w_sb = w_tiles.pop((tch, og))
                w8_sb = w8_tiles.pop((tch, og))
                if idx + 3 < len(passes):
                    load_w(passes[idx + 3])
                # Trickle the second x wave in behind the W prefetches.
                for _ in range(2):
                    if x2q:
                        kind, i = x2q.pop(0)
                        if kind == "x":
                            t = xp.tile([128, NMOV], bf16, tag="x")
                            nc.sync.dma_start(t[:], x_d[i, 1])
                            x_sb[i][1] = t
                        else:
                            t = x8p.tile([128, 2, NMOV], fp8, tag="x8")
                            nc.sync.dma_start(t[:], x8_d[i, 1])
                            x8_sb[i][1] = t
                acc = ps1.tile([128, NMOV], f32, tag="acc")
                # fp8 block strictly interior to the pass: the psum group's
                # start/stop flags both live on bf16 k-tiles, and the stage-2
                # flush (bf16) leads the pass. Near the end, flush two per
                # pass so the pending queue is nearly drained when the last
                # stage-1 block finishes.
                nflush = 2 if idx >= len(passes) - (FLUSH_LAG - 1) else 1
                for _ in range(nflush):
                    if len(pending_q) >= FLUSH_LAG or (nflush > 1 and pending_q):
                        flush_stage2(pending_q.popleft())
                bf16_mms(acc, w_sb, tch, 0, KB // 2)
                fp8_mms(acc, w8_sb, tch)
                bf16_mms(acc, w_sb, tch, KB // 2, KB)
                pending_q.append((evac_y(acc, og), og, tch))
            while pending_q:
                flush_stage2(pending_q.popleft())

    nc.compile()
    return nc


def _get_nc():
    if "nc" not in _CACHE:
        _CACHE["nc"] = _build()
    return _CACHE["nc"]


def _run(x, W, b, Wg, bg, trace=False, tmpdir=None):
    x = np.ascontiguousarray(x, dtype=np.float32)
    W = np.ascontiguousarray(W, dtype=np.float32)
    b = np.ascontiguousarray(b, dtype=np.float32)
    Wg = np.ascontiguousarray(Wg, dtype=np.float32)
    bg = np.ascontiguousarray(bg, dtype=np.float32)

    # Host-side layout prep (pure permutes + weight casts, no math).
    # x: [B,S,D] -> [core, tch, t, kt, p]; bf16 tiles kt<KB, fp8 pairs above.
    xr = x.reshape(NCORES, NCH, NMOV, KT, 128)
    x_dev = np.ascontiguousarray(
        xr[:, :, :, :KB].transpose(0, 3, 1, 4, 2).astype(ml_dtypes.bfloat16)
    )
    x8r = xr[:, :, :, KB:].reshape(NCORES, NCH, NMOV, UF8, 2, 128)
    x8_dev = np.ascontiguousarray(
        x8r.transpose(0, 3, 1, 5, 4, 2).astype(ml_dtypes.float8_e4m3)
    )
    # W: [D_out, D_in] -> [og, p, kt*128 + o] with the 2^6 pre-scale
    Ws = W * WSCALE
    wr = Ws.reshape(G, 128, KT, 128)          # [og, o, kt, p]
    w_dev = np.ascontiguousarray(
        wr[:, :, :KB].transpose(0, 3, 2, 1).reshape(G, 128, KB * 128)
        .astype(ml_dtypes.bfloat16)
    )
    w8r = wr[:, :, KB:].reshape(G, 128, UF8, 2, 128)   # [og, m, u, j, p]
    w8_dev = np.ascontiguousarray(
        w8r.transpose(0, 4, 2, 3, 1).astype(ml_dtypes.float8_e4m3)
    )
    wg_dev = np.ascontiguousarray(
        Wg.transpose(2, 0, 1).reshape(128, G * IG).astype(ml_dtypes.bfloat16)
    )
    b_dev = np.ascontiguousarray(b.reshape(G, 128).T)
    bg_dev = np.ascontiguousarray(bg.T)

    in_maps = [
        {
            "x": x_dev[c], "x8": x8_dev[c], "w": w_dev, "w8": w8_dev,
            "wg": wg_dev, "b": b_dev, "bg": bg_dev,
        }
        for c in range(NCORES)
    ]
    nc = _get_nc()
    res = bass_utils.run_bass_kernel_spmd(
        nc, in_maps, core_ids=list(range(NCORES)), trace=trace, tmpdir=tmpdir
    )
    _CACHE["last_result"] = res

    out_t = np.concatenate(
        [res.results[c]["o"].reshape(D, TPC) for c in range(NCORES)], axis=1
    )
    return np.ascontiguousarray(out_t.T).reshape(B, S, D)


def kernel(x, W, b, Wg, bg):
    return _run(x, W, b, Wg, bg, trace=False)


# revision 38
# speedup vs baseline: 1.0198x; 1.0198x over previous
"""Trainium2 Bass kernel for EnhanceLayerLinear.

Computes out = GroupedLinear(Linear(x)):
    y = x @ W.T + b                      [B,S,D]
    out[..., g, :] = y[..., g, :] @ Wg[g].T + bg[g]   (block-diagonal, G groups)

Sharding: data-parallel over tokens (B*S = 8192 -> 1024 per core). Each core
runs both GEMM stages locally; the grouped stage shards trivially since it is
applied per token.

Mixed-precision stage 1: the 4096-wide contraction is split into KB=22 bf16
k-tiles plus UF8=5 fp8e4 DoubleRow units (2 k-tiles each). A DoubleRow matmul
contracts K=256 (two k-planes packed per partition: lhsT [128,2,128], rhs
[128,2,512] -> out [128,512]) in the same ~214 ns slot as one bf16 K=128 MM
-- 2x the MAC rate, measured in-kernel -- halving stage-1 PE time on the
covered fraction. fp8 coverage is capped by the 2e-2 error gate: 10/32 of
the contraction in e4m3 (both operands) measures 1.53e-2 absmax-rel /
1.78e-2 L2 vs the fp32 reference on the full output.

fp8 scaling: W is pre-scaled by 64 before quantization (sigma(64W)=1.28 sits
mid-range in e4m3; unscaled W would land 56% of weights in subnormals). The
bf16 W tiles carry the same 2^6 factor (exact in bf16) so both dtypes
accumulate into ONE psum tile per og-pass (start on the pass's first MM,
stop on its last, skip_group_check elsewhere), and the psum evacuation
applies scale=2^-6 for free via the activation's scale operand. Pass block
order alternates by parity ([bf16,fp8] / [fp8,bf16]) so adjacent passes'
DoubleRow blocks run back-to-back, halving bf16<->DoubleRow mode switches.

Stage 2 runs in bf16 (y evacuated as bf16, Wg bf16): 213 ns/slot vs f32r's
422 ns (2-pass fp32 LDWEIGHTS cannot hide); the added y-rounding error is
~2.6e-3, inside the budget.

Layout trick (from the bf16 baseline): stage 1 computes y TRANSPOSED
(features on partitions, tokens on the free axis) so each 128-row psum tile
is exactly one group's slice with stage 2's contraction already on
partitions; the grouped matmul chains with zero on-chip transposes. The host
hands the kernel pre-transposed views of x / W / Wg and re-transposes the
output.
"""

from collections import deque

import ml_dtypes
import numpy as np

import concourse.bacc as bacc
import concourse.bass as bass
import concourse.tile as tile
from concourse import mybir
from concourse import bass_utils

f32 = mybir.dt.float32
bf16 = mybir.dt.bfloat16
fp8 = mybir.dt.float8e4
DR = mybir.MatmulPerfMode.DoubleRow
ACT_ID = mybir.ActivationFunctionType.Identity

B, S, D = 4, 2048, 4096
T = B * S                 # 8192 tokens
G, IG = 32, 128           # groups x group size (4096 = 32*128)
NCORES = 8
TPC = T // NCORES         # 1024 tokens per core
KT = D // 128             # 32 contraction tiles
UF8 = 5                   # fp8 DoubleRow units (2 k-tiles each)
KB = KT - 2 * UF8         # bf16 k-tiles
NMOV = 512                # moving free dim per bf16 matmul (= one psum bank)
NCH = TPC // NMOV         # 2 token chunks per core
WSCALE = 64.0             # fp8 weight pre-scale (power of 2)

_CACHE = {}


def _build():
    nc = bacc.Bacc("TRN2", target_bir_lowering=False, debug=False)
    # x_d[kt, tch, p, t]    = x[core_t0 + tch*512 + t, kt*128 + p]    (bf16 kt)
    # x8_d[u, tch, p, j, t] = e4m3 of x[..., (KB+2u+j)*128 + p]
    # w_d[og, p, kt*128+o]  = bf16(64*W[og*128+o, kt*128+p])
    # w8_d[og, p, u, j, m]  = e4m3(64*W[og*128+m, (KB+2u+j)*128+p])
    # wg_d[i, g*128+o] = Wg[g, o, i];  b_d[i, g] = b[g*128+i];  bg_d[o, g]
    x_d = nc.dram_tensor("x", [KB, NCH, 128, NMOV], bf16, kind="ExternalInput")
    x8_d = nc.dram_tensor(
        "x8", [UF8, NCH, 128, 2, NMOV], fp8, kind="ExternalInput"
    )
    w_d = nc.dram_tensor("w", [G, 128, KB * 128], bf16, kind="ExternalInput")
    w8_d = nc.dram_tensor("w8", [G, 128, UF8, 2, 128], fp8, kind="ExternalInput")
    wg_d = nc.dram_tensor("wg", [128, G * IG], bf16, kind="ExternalInput")
    b_d = nc.dram_tensor("b", [128, G], f32, kind="ExternalInput")
    bg_d = nc.dram_tensor("bg", [128, G], f32, kind="ExternalInput")
    # o_d[og, o, t] = out[core_t0 + t, og*128 + o]                (outT)
    o_d = nc.dram_tensor("o", [G, 128, TPC], f32, kind="ExternalOutput")

    with tile.TileContext(nc) as tc:
        with (
            tc.tile_pool(name="xp", bufs=KB * NCH) as xp,
            tc.tile_pool(name="x8p", bufs=UF8 * NCH) as x8p,
            tc.tile_pool(name="wp", bufs=6) as wp,
            tc.tile_pool(name="w8p", bufs=6) as w8p,
            tc.tile_pool(name="cp", bufs=1) as cp,
            tc.tile_pool(name="yp", bufs=18) as yp,
            tc.tile_pool(name="op", bufs=6) as op,
            tc.tile_pool(name="ps1", bufs=4, space=bass.MemorySpace.PSUM) as ps1,
            tc.tile_pool(name="ps2", bufs=4, space=bass.MemorySpace.PSUM) as ps2,
        ):
            w_tiles = {}
            w8_tiles = {}

            # DMA queue split (queue = issuing engine): x wave rides the sync
            # HW-DGE queue, the W stream rides gpsimd's queue, outputs and
            # consts ride scalar's -- three streams in parallel instead of
            # one serialized sync queue.
            def load_w(key):
                t = wp.tile([128, KB * 128], bf16, tag="w")
                nc.gpsimd.dma_start(t[:], w_d[key[1]])
                w_tiles[key] = t
                t8 = w8p.tile([128, UF8, 2, 128], fp8, tag="w8")
                nc.gpsimd.dma_start(t8[:], w8_d[key[1]])
                w8_tiles[key] = t8

            # The first ~25us is DMA-bandwidth-bound, so queue order here IS
            # the schedule. The first RAMP groups run INTERLEAVED (kt-major
            # across RAMP psum banks) so each arriving x tile feeds RAMP
            # matmuls and the PE stays busy through the whole x wave; their W
            # tiles are delivered as just-in-time column chunks between the x
            # tiles they gate. The fp8 halves of the ramp groups run last,
            # fed by the small x8/w8 tiles queued behind the bf16 x wave.
            RAMP = 4
            b_sb = cp.tile([128, G], f32)
            nc.scalar.dma_start(b_sb[:], b_d[:])
            ramp_w = []
            ramp_w8 = []
            for og in range(RAMP):
                t = wp.tile([128, KB * 128], bf16, tag="w")
                ramp_w.append(t)
                w_tiles[(0, og)] = t
                t8 = w8p.tile([128, UF8, 2, 128], fp8, tag="w8")
                ramp_w8.append(t8)
                w8_tiles[(0, og)] = t8
            x_sb = [[None] * NCH for _ in range(KB)]
            x8_sb = [[None] * NCH for _ in range(UF8)]
            wg_sb = cp.tile([128, G * IG], bf16)
            bg_sb = cp.tile([128, G], f32)
            WCHUNKS = [1, 5, 5, 6, 5]
            kt0 = 0
            for wc in WCHUNKS:
                lo, hi = kt0 * 128, (kt0 + wc) * 128
                for og in range(RAMP):
                    nc.gpsimd.dma_start(ramp_w[og][:, lo:hi], w_d[og][:, lo:hi])
                for kt in range(kt0, kt0 + wc):
                    t = xp.tile([128, NMOV], bf16, tag="x")
                    nc.sync.dma_start(t[:], x_d[kt, 0])
                    x_sb[kt][0] = t
                kt0 += wc
            for og in range(RAMP):
                nc.gpsimd.dma_start(ramp_w8[og][:], w8_d[og])
            for u in range(UF8):
                t = x8p.tile([128, 2, NMOV], fp8, tag="x8")
                nc.sync.dma_start(t[:], x8_d[u, 0])
                x8_sb[u][0] = t
            load_w((0, RAMP))
            load_w((0, RAMP + 1))
            load_w((0, RAMP + 2))
            nc.scalar.dma_start(wg_sb[:], wg_d[:])
            nc.scalar.dma_start(bg_sb[:], bg_d[:])

            pending_q = deque()
            FLUSH_LAG = 6

            def flush_stage2(p):
                y_sb, og2, tch2 = p
                acc2 = ps2.tile([128, NMOV], f32, tag="acc2")
                nc.tensor.matmul(
                    acc2[:],
                    wg_sb[:, og2 * IG:(og2 + 1) * IG],
                    y_sb[:],
                    start=True,
                    stop=True,
                )
                o_sb = op.tile([128, NMOV], f32, tag="o")
                nc.scalar.activation(
                    o_sb[:], acc2[:], ACT_ID, bias=bg_sb[:, og2:og2 + 1]
                )
                nc.scalar.dma_start(
                    o_d[og2][:, tch2 * NMOV:(tch2 + 1) * NMOV], o_sb[:]
                )

            def fp8_mms(acc, w8_sb, tch):
                # One DoubleRow MM per unit: rhs free = 2x512 -> out free 512,
                # so the fp8 MMs write the same full psum width as the bf16
                # ones, at ~214 ns each vs ~428 ns for the bf16 k-tile PAIR
                # they replace (2x MAC rate, measured in-kernel). The block
                # sits strictly interior to the pass (start/stop both live on
                # bf16 k-tiles).
                for u in range(UF8):
                    nc.tensor.matmul(
                        acc[:],
                        w8_sb[:, u],
                        x8_sb[u][tch][:],
                        start=False,
                        stop=False,
                        perf_mode=DR,
                        skip_group_check=True,
                    )

            def bf16_mms(acc, w_sb, tch, lo, hi):
                for kt in range(lo, hi):
                    nc.tensor.matmul(
                        acc[:],
                        w_sb[:, kt * 128:(kt + 1) * 128],
                        x_sb[kt][tch][:],
                        start=(kt == 0),
                        stop=(kt == KB - 1),
                        skip_group_check=True,
                    )

            def evac_y(acc, og):
                y_sb = yp.tile([128, NMOV], bf16, tag="y")
                nc.scalar.activation(
                    y_sb[:], acc[:], ACT_ID,
                    bias=b_sb[:, og:og + 1], scale=1.0 / WSCALE,
                )
                return y_sb

            # Interleaved ramp: RAMP accumulation groups advance together,
            # kt-major, one psum bank each, paced by the x-tile arrivals;
            # every group's fp8 block runs at the end (one mode switch for
            # the whole ramp), fed by the small x8/w8 tiles queued behind
            # the bf16 x wave.
            accs = []
            for _r in range(RAMP):
                acc_r = ps1.tile([128, NMOV], f32, tag="acc")
                accs.append(acc_r)
            for kt in range(KB - 1):
                for og in range(RAMP):
                    nc.tensor.matmul(
                        accs[og][:],
                        ramp_w[og][:, kt * 128:(kt + 1) * 128],
                        x_sb[kt][0][:],
                        start=(kt == 0),
                        stop=False,
                        skip_group_check=True,
                    )
            for og in range(RAMP):
                fp8_mms(accs[og], ramp_w8[og], 0)
                nc.tensor.matmul(
                    accs[og][:],
                    ramp_w[og][:, (KB - 1) * 128:KB * 128],
                    x_sb[KB - 1][0][:],
                    start=False,
                    stop=True,
                    skip_group_check=True,
                )
                pending_q.append((evac_y(accs[og], og), og, 0))

            # tch outer: the whole first token-chunk pass (32 groups) runs
            # before any tch=1 tile is needed, so the second x wave has
            # enormous DMA slack. W streams twice; still far below the
            # per-core HBM budget.
            x2q = [("x", kt) for kt in range(KB)] + [("x8", u) for u in range(UF8)]
            passes = [(tch, og) for tch in range(NCH) for og in range(G)]
            for idx in range(RAMP, len(passes)):
                tch, og = passes[idx]
                w_sb = w_tiles.pop((tch, og))
                w8_sb = w8_tiles.pop((tch, og))
                if idx + 3 < len(passes):
                    load_w(passes[idx + 3])
                # Trickle the second x wave in behind the W prefetches.
                for _ in range(2):
                    if x2q:
                        kind, i = x2q.pop(0)
                        if kind == "x":
                            t = xp.tile([128, NMOV], bf16, tag="x")
                            nc.sync.dma_start(t[:], x_d[i, 1])
                            x_sb[i][1] = t
                        else:
                            t = x8p.tile([128, 2, NMOV], fp8, tag="x8")
                            nc.sync.dma_start(t[:], x8_d[i, 1])
                            x8_sb[i][1] = t
                acc = ps1.tile([128, NMOV], f32, tag="acc")
                # fp8 block strictly interior to the pass: the psum group's
                # start/stop flags both live on bf16 k-tiles, and the stage-2
                # flush (bf16) leads the pass. Near the end, flush two per
                # pass so the pending queue is nearly drained when the last
                # stage-1 block finishes.
                nflush = 2 if idx >= len(passes) - (FLUSH_LAG - 1) else 1
                for _ in range(nflush):
                    if len(pending_q) >= FLUSH_LAG or (nflush > 1 and pending_q):
                        flush_stage2(pending_q.popleft())
                bf16_mms(acc, w_sb, tch, 0, KB // 2)
                fp8_mms(acc, w8_sb, tch)
                bf16_mms(acc, w_sb, tch, KB // 2, KB)
                pending_q.append((evac_y(acc, og), og, tch))
            while pending_q:
                flush_stage2(pending_q.popleft())

    nc.compile()
    return nc


def _get_nc():
    if "nc" not in _CACHE:
        _CACHE["nc"] = _build()
    return _CACHE["nc"]


def _run(x, W, b, Wg, bg, trace=False, tmpdir=None):
    x = np.ascontiguousarray(x, dtype=np.float32)
    W = np.ascontiguousarray(W, dtype=np.float32)
    b = np.ascontiguousarray(b, dtype=np.float32)
    Wg = np.ascontiguousarray(Wg, dtype=np.float32)
    bg = np.ascontiguousarray(bg, dtype=np.float32)

    # Host-side layout prep (pure permutes + weight casts, no math).
    # x: [B,S,D] -> [core, tch, t, kt, p]; bf16 tiles kt<KB, fp8 pairs above.
    xr = x.reshape(NCORES, NCH, NMOV, KT, 128)
    x_dev = np.ascontiguousarray(
        xr[:, :, :, :KB].transpose(0, 3, 1, 4, 2).astype(ml_dtypes.bfloat16)
    )
    x8r = xr[:, :, :, KB:].reshape(NCORES, NCH, NMOV, UF8, 2, 128)
    x8_dev = np.ascontiguousarray(
        x8r.transpose(0, 3, 1, 5, 4, 2).astype(ml_dtypes.float8_e4m3)
    )
    # W: [D_out, D_in] -> [og, p, kt*128 + o] with the 2^6 pre-scale
    Ws = W * WSCALE
    wr = Ws.reshape(G, 128, KT, 128)          # [og, o, kt, p]
    w_dev = np.ascontiguousarray(
        wr[:, :, :KB].transpose(0, 3, 2, 1).reshape(G, 128, KB * 128)
        .astype(ml_dtypes.bfloat16)
    )
    w8r = wr[:, :, KB:].reshape(G, 128, UF8, 2, 128)   # [og, m, u, j, p]
    w8_dev = np.ascontiguousarray(
        w8r.transpose(0, 4, 2, 3, 1).astype(ml_dtypes.float8_e4m3)
    )
    wg_dev = np.ascontiguousarray(
        Wg.transpose(2, 0, 1).reshape(128, G * IG).astype(ml_dtypes.bfloat16)
    )
    b_dev = np.ascontiguousarray(b.reshape(G, 128).T)
    bg_dev = np.ascontiguousarray(bg.T)

    in_maps = [
        {
            "x": x_dev[c], "x8": x8_dev[c], "w": w_dev, "w8": w8_dev,
            "wg": wg_dev, "b": b_dev, "bg": bg_dev,
        }
        for c in range(NCORES)
    ]
    nc = _get_nc()
    res = bass_utils.run_bass_kernel_spmd(
        nc, in_maps, core_ids=list(range(NCORES)), trace=trace, tmpdir=tmpdir
    )
    _CACHE["last_result"] = res

    out_t = np.concatenate(
        [res.results[c]["o"].reshape(D, TPC) for c in range(NCORES)], axis=1
    )
    return np.ascontiguousarray(out_t.T).reshape(B, S, D)


def kernel(x, W, b, Wg, bg):
    return _run(x, W, b, Wg, bg, trace=False)
